# revision 2
# baseline (speedup 1.0000x reference)
"""CRF loss (log_z - gold_score) on 8 Trainium2 NeuronCores — v2.

Strategy (data-parallel over batch):
  - Shard the 1024-item batch as 128 contiguous items per core, folded as
    [128 part = tag j + 64*h, 64 cols = items within half].
  - log Z via a probability-domain FORWARD chain (256 steps) and an
    independent BACKWARD chain (256 steps) run concurrently:
      fwd:  alpha_t = (E^T alpha_{t-1}) * F_t
      bwd:  g_t = (E g_{t+1}) * F_t
      Z_b  = alpha_255 . beta_255,  beta_255 = E g_256
    Halves the sequential depth (2 independent chains hide engine latency).
  - Only DVE may read PSUM (GPSIMD may not), so each depth-step's fwd and
    bwd matmul outputs land in ONE shared PSUM tile and a single wide DVE
    multiply [128, 64] (fwd32 || bwd32 per column-stream) serves both
    chains — amortizing DVE's fixed PSUM-access penalty.
  - No measured renormalization: the host subtracts a constant C=ln(64)+0.5
    (the expected per-step log growth) from every real step's emissions, so
    chain values random-walk near 1.0 well inside bf16 range. Exactly
    accounted: ln Z_b += C * len_b on the host.
  - Masking is baked into the emissions on the host (collapse-to-tag-0 pad
    trick): pads multiply the state by exactly 1 (growth-compensated).
  - F = exp(emit' - C) is precomputed on the host (bf16) — no device exp.
    strans is folded into F_0.
  - Final reduction ON DEVICE: z = alpha*beta (DVE), Z = colsum via ones
    matmul (PE), ACT Ln with accum_out -> [2,1] f32 = sum_b ln Zraw_b.
    Output is 8 bytes per core.
  - Gold path score is O(L*B) gather bookkeeping done on the host.
"""

import sys
from contextlib import ExitStack

import numpy as np

sys.path.insert(0, "/opt/trn_rl_repo")

import ml_dtypes  # noqa: E402
import concourse.bass as bass  # noqa: E402
import concourse.tile as tile  # noqa: E402
from concourse import bacc, mybir  # noqa: E402
from concourse.bass_utils import run_bass_kernel_spmd  # noqa: E402

BF16 = ml_dtypes.bfloat16

L, B, T, NC = 512, 1024, 64, 8
HALF = L // 2                  # 256 depth steps per direction
CH = 16                        # steps per DMA chunk
NCH = HALF // CH               # 16 chunks
C_RENORM = np.float32(np.log(64.0) + 0.5)
NEG = np.float32(-1e30)

_CACHE = {}


def _build_nc():
    f32 = mybir.dt.float32
    bf = mybir.dt.bfloat16
    nc = bacc.Bacc("TRN2", target_bir_lowering=False, debug=False)
    # combined chunks: per step the 128 free cols are
    # [fwd cols 0:32 | bwd cols 0:32 | fwd cols 32:64 | bwd cols 32:64]
    fc_d = nc.dram_tensor("fc", [NCH, 128, CH * 128], bf, kind="ExternalInput")
    e2_d = nc.dram_tensor("e2", [128, 128], bf, kind="ExternalInput")
    e2t_d = nc.dram_tensor("e2t", [128, 128], bf, kind="ExternalInput")
    onesbd_d = nc.dram_tensor("onesbd", [128, 2], bf, kind="ExternalInput")
    acc_d = nc.dram_tensor("acc", [2, 1], f32, kind="ExternalOutput")

    with tile.TileContext(nc) as tc, ExitStack() as ctx:
        cpool = ctx.enter_context(tc.tile_pool(name="consts", bufs=1))
        fcpool = ctx.enter_context(tc.tile_pool(name="fcch", bufs=1))
        upool = ctx.enter_context(tc.tile_pool(name="u", bufs=1, space="PSUM"))
        fpool = ctx.enter_context(tc.tile_pool(name="bfin", bufs=1, space="PSUM"))
        fin = ctx.enter_context(tc.tile_pool(name="fin", bufs=1, space="PSUM"))

        E2 = cpool.tile([128, 128], bf, tag="E2")
        nc.sync.dma_start(E2[:], e2_d[:])
        E2T = cpool.tile([128, 128], bf, tag="E2T")
        nc.sync.dma_start(E2T[:], e2t_d[:])
        ones = cpool.tile([128, 2], bf, tag="ones")
        nc.sync.dma_start(ones[:], onesbd_d[:])

        fcs = []
        for ci in range(NCH):
            t = fcpool.tile([128, CH * 128], bf, tag=f"fc{ci}")
            nc.sync.dma_start(t[:], fc_d[ci])
            fcs.append(t)

        # ag: [alpha s0 | g s0 | alpha s1 | g s1], 32 cols each
        ag = cpool.tile([128, 128], bf, tag="ag")

        def fsl(k, s):  # f pair (fwd||bwd) for depth-step k, stream s
            ci, st = divmod(k, CH)
            return fcs[ci][:, 128 * st + 64 * s : 128 * st + 64 * (s + 1)]

        for k in range(1, HALF):
            us = []
            for s in (0, 1):  # fwd matmuls (stationary E2)
                u = upool.tile([128, 64], f32, tag=f"u{s}")
                movF = ag[:, 64 * s : 64 * s + 32] if k > 1 else fsl(0, s)[:, 0:32]
                nc.tensor.matmul(u[:, 0:32], E2[:], movF)
                us.append(u)
            for s in (0, 1):  # bwd matmuls (stationary E2T)
                movB = ag[:, 64 * s + 32 : 64 * s + 64] if k > 1 else fsl(0, s)[:, 32:64]
                nc.tensor.matmul(us[s][:, 32:64], E2T[:], movB)
            for s in (0, 1):  # one wide DVE multiply per stream serves both chains
                nc.vector.tensor_tensor(
                    ag[:, 64 * s : 64 * (s + 1)], us[s][:], fsl(k, s),
                    mybir.AluOpType.mult,
                )

        # beta_255 = E g_256 ; z = alpha_255 * beta_255 ; Z = colsums(z)
        z = cpool.tile([128, 64], bf, tag="z")
        for s in (0, 1):
            b = fpool.tile([128, 32], f32, tag=f"bf{s}")
            nc.tensor.matmul(b[:], E2T[:], ag[:, 64 * s + 32 : 64 * s + 64])
            nc.vector.tensor_tensor(
                z[:, 32 * s : 32 * (s + 1)], b[:], ag[:, 64 * s : 64 * s + 32],
                mybir.AluOpType.mult,
            )
        Z = fin.tile([2, 64], f32, tag="Z")
        nc.tensor.matmul(Z[:], ones[:], z[:])
        lnz = cpool.tile([2, 64], f32, tag="lnz")
        acc = cpool.tile([2, 1], f32, tag="acc")
        nc.scalar.activation(
            lnz[:], Z[:], mybir.ActivationFunctionType.Ln, accum_out=acc[:]
        )
        nc.sync.dma_start(acc_d[:], acc[:])

    nc.compile()
    return nc


def _prepare_host(emit, trans, strans, etrans, mask):
    lens = mask.sum(0).astype(np.int64)  # [B], all >= 1 (mask[0] all True)
    ar = np.arange(B)
    C = float(C_RENORM)
    emitP = emit.astype(np.float32).copy()
    # fold end transition into the last real step
    emitP[lens - 1, ar, :] += (etrans - trans[:, 0])[None, :]
    # fold start transition into step 0
    emitP[0] += strans[None, :]
    # pad steps: -inf except tag 0 (growth-compensated so pads multiply by 1)
    tgrid = np.arange(L)[:, None]
    padmask = tgrid >= lens[None, :]
    emitP[padmask] = NEG
    short = lens < L
    emitP[lens[short], ar[short], 0] = C  # first pad collapses into tag 0
    laterpad = tgrid > lens[None, :]
    e0 = emitP[:, :, 0]
    e0[laterpad] = -trans[0, 0] + C
    F = np.exp(emitP - C).astype(BF16)  # [L, B, T] bf16

    E = np.exp(trans.astype(np.float32))
    E2 = np.zeros((128, 128), np.float32)
    E2[:64, :64] = E
    E2[64:, 64:] = E
    E2T = np.ascontiguousarray(E2.T).astype(BF16)
    E2 = E2.astype(BF16)

    onesbd = np.zeros((128, 2), np.float32)
    onesbd[:64, 0] = 1.0
    onesbd[64:, 1] = 1.0
    onesbd = onesbd.astype(BF16)

    in_maps = []
    for c in range(NC):
        Fc = F[:, 128 * c : 128 * (c + 1), :]  # [L, 128, 64] (t, b_local, j)
        v = Fc.reshape(L, 2, 64, T).transpose(0, 1, 3, 2)  # [t, h, j, b']
        Ffold = np.ascontiguousarray(v).reshape(L, 128, 64)
        ffwd = Ffold[:HALF]                       # t = 0..255
        fbwd = Ffold[L - 1 : HALF - 1 : -1]       # t = 511..256
        comb = np.concatenate(
            [ffwd[:, :, 0:32], fbwd[:, :, 0:32], ffwd[:, :, 32:64], fbwd[:, :, 32:64]],
            axis=2,
        )  # [HALF, 128, 128]
        fc = np.ascontiguousarray(
            comb.reshape(NCH, CH, 128, 128).transpose(0, 2, 1, 3)
        ).reshape(NCH, 128, CH * 128)
        in_maps.append({"fc": fc, "e2": E2, "e2t": E2T, "onesbd": onesbd})
    return in_maps, lens


def _gold_score(emit, trans, strans, etrans, target, mask, lens):
    target = target.astype(np.int64)
    emit_sc = np.take_along_axis(emit, target[:, :, None], axis=2)[..., 0]
    trans_sc = np.concatenate(
        [np.zeros((1, B), np.float32), trans[target[:-1], target[1:]]], axis=0
    )
    score = np.where(mask, emit_sc + trans_sc, np.float32(0.0)).sum(dtype=np.float64)
    score = score + strans[target[0]].sum(dtype=np.float64)
    last_tag = target[lens - 1, np.arange(B)]
    score = score + etrans[last_tag].sum(dtype=np.float64)
    return score / float(B)


def kernel(emit, trans, strans, etrans, target, mask):
    emit = np.asarray(emit, np.float32)
    trans = np.asarray(trans, np.float32)
    strans = np.asarray(strans, np.float32)
    etrans = np.asarray(etrans, np.float32)
    mask_b = np.asarray(mask).astype(bool)

    in_maps, lens = _prepare_host(emit, trans, strans, etrans, mask_b)

    if "nc" not in _CACHE:
        _CACHE["nc"] = _build_nc()
    nc = _CACHE["nc"]
    res = run_bass_kernel_spmd(nc, in_maps, core_ids=list(range(NC)))

    total = 0.0
    for c in range(NC):
        acc = np.asarray(res.results[c]["acc"], np.float64)  # [2, 1]
        total += acc.sum()
    log_z = (total + float(C_RENORM) * float(lens.sum())) / float(B)

    gold = _gold_score(emit, trans, strans, etrans, np.asarray(target), mask_b, lens)
    return np.asarray(log_z - gold, dtype=np.float32)
